# revision 32
# baseline (speedup 1.0000x reference)
"""DeepSeekV2-style MLA decode attention (MQA, B=128 decode tokens) on 8 trn2 NeuronCores.

v10 (verified 271145 ns, rel err 0.018808): head-sharded Wq/Wo + batch-sharded
attention with two AllToAlls. DMA priority enforced via real data deps (Tile
schedules by deps, not program order): kv ring gated behind the last wq tile's
DMA, wo prefetch gated behind the kv prefetch and the qj gathers, all via
dummy pool-buffer writes. Wq in fp8-e3m4 (x64 scale, h staged /64, clip +-15).
Warm-up AllToAll absorbs the first-collective Mesh setup. wq/hts pools scoped
to phase A so the 32-tile wo stream reuses their SBUF. kv cache as one
host-staged blob ([kt | vv+mask] 16-tile groups, ~1MB DMAs); mask column
doubles as the softmax denominator via the PV matmul.
"""

import os
import numpy as np
import ml_dtypes

import concourse.bass as bass
import concourse.mybir as mybir
import concourse.tile as tile
from concourse import bacc
from concourse.bass_utils import run_bass_kernel_spmd

BF16 = ml_dtypes.bfloat16
P = 128
B, MAX_S, HID = 128, 4096, 5120
H, D, D_ROPE, D_V = 128, 128, 64, 128
NC, HPC, BPC = 8, 16, 16           # cores, heads/core, slots(batches)/core
QCOLS = HPC * D + 256              # 2304: per-core q cols + kv cols
SCALE = float(D) ** -0.5
PROJ_CHUNKS = [(0, 512), (512, 512), (1024, 512), (1536, 512), (2048, 256)]
VW = D_V + 1                       # 129: v tile width incl. mask column
TW = P + VW                        # 257: cols per s-tile in the kv blob
GRP = 16                           # s-tiles per kv DMA group (~1MB transfers)
SUB = 4                            # s-tiles per exp/psum sub-chunk
NK16 = 0                           # bf16 wq super-tiles (2 k-tiles each)
NK8 = 20                           # fp8-e3m4 wq super-tiles (2 k-tiles each)
GQ = 64.0                          # wq stage scale (h staged /GQ)
WO_HALF = HID // 2                 # 2560
KV_PRE = 6                         # kv ring depth (groups)
WO_PRE = 22                        # wo ring depth (tiles; reuses wq space)

dtb = mybir.dt.bfloat16
dtf = mybir.dt.float32
dt8 = mybir.dt.float8e3
E3M4 = ml_dtypes.float8_e3m4

_PROGRAM_CACHE: dict = {}
LAST_RESULTS = None


def _install_ntff_hook():
    import sys
    import types
    try:
        import antenv.axon_hooks  # noqa: F401
        return
    except ImportError:
        pass
    try:
        from trn_agent_boot.trn_boot import _ntff_profile_via_ctypes
        hook = _ntff_profile_via_ctypes("/opt/axon/libaxon_pjrt.so")
        mod = types.ModuleType("antenv.axon_hooks")
        mod._hook = hook
        mod.set_axon_ntff_profile_hook = lambda h: setattr(mod, "_hook", h)
        mod.get_axon_ntff_profile_hook = lambda: mod._hook
        sys.modules["antenv.axon_hooks"] = mod
        import antenv
        antenv.axon_hooks = mod
    except Exception:
        pass


def _groups_of(Tj, off0):
    """[(t0, w, off), ...] groups of up to GRP s-tiles; off = blob col offset."""
    out = []
    t, off = 0, off0
    while t < Tj:
        w = min(GRP, Tj - t)
        out.append((t, w, off))
        off += w * TW
        t += w
    return out


def _build_program(T, kvoff, KVCOLS, do_compile=True):
    """Build + compile the per-core bass program. T: per-slot tile counts;
    kvoff[j]: [(t0, w, off)] groups of slot j in the kv blob."""
    nc = bacc.Bacc("TRN2", target_bir_lowering=False, debug=False, num_devices=NC)

    ht_d = nc.dram_tensor("ht", [P, HID], dtb, kind="ExternalInput")
    wq8_d = nc.dram_tensor("wq8", [P, NK8 * 2 * QCOLS], dt8, kind="ExternalInput")
    kv_d = nc.dram_tensor("kv", [P, KVCOLS], dtb, kind="ExternalInput")
    wo_d = nc.dram_tensor("wo", [HPC * D_V, HID], dtb, kind="ExternalInput")
    idn_d = nc.dram_tensor("idn", [P, P], dtb, kind="ExternalInput")
    out_d = nc.dram_tensor("outp", [P, HID], dtb, kind="ExternalOutput")

    rg = [list(range(NC))]
    Exp = mybir.ActivationFunctionType.Exp

    from contextlib import ExitStack
    wo_stack = ExitStack()
    with tile.TileContext(nc) as tc, wo_stack:
        with (
            tc.tile_pool(name="cpool", bufs=1) as cpool,
            tc.tile_pool(name="kvpool", bufs=KV_PRE) as kvpool,
            tc.tile_pool(name="qtpool", bufs=BPC) as qtpool,
            tc.tile_pool(name="epool", bufs=4) as epool,
            tc.tile_pool(name="spool", bufs=2) as spool,
            tc.tile_pool(name="lhpool", bufs=HPC) as lhpool,
            tc.tile_pool(name="opool", bufs=2) as opool,
            tc.tile_pool(name="dram", bufs=1, space="DRAM") as dram,
        ):
            # ---- constants / global loads (sync ring: first) ----
            idn = cpool.tile([P, P], dtb)
            nc.sync.dma_start(idn[:], idn_d.ap())

            qcc_in = dram.tile([P, QCOLS], dtb)
            qcc_out = dram.tile([P, QCOLS], dtb)
            acc_in = dram.tile([P, BPC * D_V], dtb)
            acc_out = dram.tile([P, BPC * D_V], dtb)

            # tiny warm-up AllToAll: absorbs the first-collective Mesh setup
            # (~11.5us trigger delay) into the idle preamble window.
            wrm_in = dram.tile([P, 16], dtb)
            wrm_out = dram.tile([P, 16], dtb)
            nc.gpsimd.collective_compute(
                "AllToAll", mybir.AluOpType.bypass, replica_groups=rg,
                ins=[wrm_in.opt()], outs=[wrm_out.opt()],
            )

            # ---- phase A: q/kv projection (this core's 16 heads, all 128 batches)
            # all 40 k-tiles fp8-e3m4; accumulate into one psum.
            # wq pools + hts live in a scope that closes after phase A so the
            # wo pool below can reuse their SBUF (Tile adds the WAR sync).
            qc_sb = cpool.tile([P, QCOLS], dtb)
            with (
                tc.tile_pool(name="wq8pool", bufs=6) as wq8pool,
                tc.tile_pool(name="htpool", bufs=1) as htpool,
            ):
                hts = htpool.tile([P, HID], dtb)
                nc.sync.dma_start(hts[:], ht_d.ap())
                wt_last = None
                with tc.tile_pool(name="pjp", bufs=1, space="PSUM") as pjp:
                    q_ps = pjp.tile([P, QCOLS], dtf)
                    for s in range(NK16 + NK8):
                        wt = wq8pool.tile([P, 2 * QCOLS], dt8, tag="wt8")
                        nc.sync.dma_start(
                            wt[:], wq8_d.ap()[:, s * 2 * QCOLS:(s + 1) * 2 * QCOLS])
                        wt_last = wt
                        for half in range(2):
                            k = 2 * s + half
                            for (c0, cw) in PROJ_CHUNKS:
                                nc.tensor.matmul(
                                    q_ps[:, c0:c0 + cw],
                                    lhsT=hts[:, k * P:(k + 1) * P],
                                    rhs=wt[:, half * QCOLS + c0:half * QCOLS + c0 + cw],
                                    start=(k == 0),
                                    stop=(k == 2 * (NK16 + NK8) - 1),
                                )
                    # copy per chunk so the tail overlaps the last matmuls
                    for (c0, cw) in PROJ_CHUNKS:
                        nc.vector.tensor_copy(qc_sb[:, c0:c0 + cw], q_ps[:, c0:c0 + cw])

                # kv ring gated behind the last wq DMA: Tile schedules by data
                # deps, so priority is enforced by writing a dummy byte (dep on
                # wq's last tile DMA) into each ring buffer before its first
                # real load. Emitted inside this scope (reads wt_last).
                for _ in range(KV_PRE):
                    dmy = kvpool.tile([P, GRP * TW], dtb, tag="kv")
                    nc.vector.tensor_copy(dmy[0:1, 0:1], wt_last[0:1, 0:1])

            # ---- kv blob loads (sync ring) ----
            kv_tiles = []
            for j in range(BPC):
                for (t0, w, off) in kvoff[j]:
                    kvt = kvpool.tile([P, GRP * TW], dtb, tag="kv")
                    nc.sync.dma_start(kvt[:, 0:w * TW], kv_d.ap()[:, off:off + w * TW])
                    kv_tiles.append(kvt)

            # ---- A2A1 staging (ACT ring) + wo prefetch (gpsimd ring) ----
            nc.scalar.dma_start(qcc_in[:], qc_sb[:])
            # wo tiles 0-13 (pool A) gated behind the kv prefetch; tiles
            # 14-21 (pool B, emitted after the qj gathers) gated behind those
            # gathers so they don't starve phase B's start; tiles 22-31 reuse
            # pool A buffers (paced by phase C consumption).
            wopoolA = wo_stack.enter_context(tc.tile_pool(name="wopoolA", bufs=14))
            wopoolB = wo_stack.enter_context(tc.tile_pool(name="wopoolB", bufs=8))
            for i in range(14):
                dmy = wopoolA.tile([P, WO_HALF], dtb, tag="wt2")
                nc.vector.tensor_copy(dmy[0:1, 0:1], kv_tiles[KV_PRE - 1][0:1, 0:1])
            wo_tiles = []
            for kt in range(14):
                half, k = divmod(kt, HPC)
                wt2 = wopoolA.tile([P, WO_HALF], dtb, tag="wt2")
                nc.gpsimd.dma_start(
                    wt2[:], wo_d.ap()[k * P:(k + 1) * P, half * WO_HALF:(half + 1) * WO_HALF])
                wo_tiles.append(wt2)

            nc.gpsimd.collective_compute(
                "AllToAll", mybir.AluOpType.bypass, replica_groups=rg,
                ins=[qcc_in.opt()], outs=[qcc_out.opt()],
            )

            # ---- phase B gathers (ACT ring: idle at this point) ----
            # qj_all[(c,hl), (j,d)] = qcc_out[c*16+j, hl*128+d]
            qj_all = cpool.tile([P, BPC * P], dtb)
            for cc in range(NC):
                nc.scalar.dma_start(
                    qj_all[cc * HPC:(cc + 1) * HPC, :]
                    .rearrange("hl (j d) -> hl j d", d=P),
                    qcc_out.rearrange("(c j) (hl d) -> c hl j d", j=BPC, d=P)[cc, 0:HPC],
                )
            # wo tiles 14-21: gated on all qj gathers (col 0 spans all blocks)
            for i in range(8):
                dmy = wopoolB.tile([P, WO_HALF], dtb, tag="wt3")
                nc.vector.tensor_copy(dmy[:, 0:1], qj_all[:, 0:1])
            for kt in range(14, 22):
                half, k = divmod(kt, HPC)
                wt2 = wopoolB.tile([P, WO_HALF], dtb, tag="wt3")
                nc.gpsimd.dma_start(
                    wt2[:], wo_d.ap()[k * P:(k + 1) * P, half * WO_HALF:(half + 1) * WO_HALF])
                wo_tiles.append(wt2)

            knew = cpool.tile([BPC, P], dtb)
            nc.scalar.dma_start(knew[:], qcc_out[0:BPC, HPC * D:HPC * D + P])
            vnew = cpool.tile([1, BPC * P], dtb)
            nc.scalar.dma_start(
                vnew.rearrange("one (j d) -> one j d", d=P),
                qcc_out.rearrange("(c j) f -> c j f", j=BPC)[0:1, :, HPC * D + P:QCOLS],
            )

            # ---- phase B: attention over this core's 16 batches ----
            attn_cc = cpool.tile([P, BPC * D_V], dtb)
            with (
                tc.tile_pool(name="tps", bufs=2, space="PSUM") as tps,
                tc.tile_pool(name="scps", bufs=3, space="PSUM") as scps,
                tc.tile_pool(name="aps", bufs=2, space="PSUM") as aps,
            ):
                knewT_ps = tps.tile([P, BPC], dtb, tag="tps")
                nc.tensor.transpose(knewT_ps[:], knew[:], idn[0:BPC, 0:BPC])
                knewT = cpool.tile([P, BPC], dtb)
                nc.vector.tensor_copy(knewT[:], knewT_ps[:])

                # hoist all qjT transposes so per-slot matmuls never wait on
                # the transpose+copy chain (qtpool holds all 16)
                qjTs = []
                for j in range(BPC):
                    qjT_ps = tps.tile([P, P], dtb, tag="tps")
                    nc.tensor.transpose(qjT_ps[:], qj_all[:, j * P:(j + 1) * P], idn[:])
                    qjT = qtpool.tile([P, P], dtb, tag="qjT")
                    nc.vector.tensor_copy(qjT[:], qjT_ps[:])
                    qjTs.append(qjT)

                gi = 0
                for j in range(BPC):
                    Tj = int(T[j])
                    qjT = qjTs[j]

                    attn_ps = aps.tile([P, VW], dtf, tag="attn")
                    for (t0, w, off) in kvoff[j]:
                        kvt = kv_tiles[gi]
                        gi += 1
                        if t0 == 0:
                            # patch new-token k (column 0 of kt) and v (row 0
                            # of vv values; its mask col is host-staged 1.0)
                            nc.vector.tensor_copy(kvt[:, 0:1], knewT[:, j:j + 1])
                            nc.vector.tensor_copy(
                                kvt[0:1, w * P:w * P + D_V],
                                vnew[0:1, j * P:j * P + D_V],
                            )
                        for u0 in range(0, w, SUB):
                            u = min(SUB, w - u0)
                            sc = scps.tile([P, SUB * P], dtf, tag="sc")
                            for li in range(u):
                                nc.tensor.matmul(
                                    sc[:, li * P:(li + 1) * P],
                                    lhsT=kvt[:, (u0 + li) * P:(u0 + li + 1) * P],
                                    rhs=qjT[:],
                                    start=True, stop=True,
                                )
                            et = epool.tile([P, SUB * P], dtb, tag="et")
                            nc.scalar.activation(
                                et[:, 0:u * P], sc[:, 0:u * P], Exp,
                                bias=0.0, scale=SCALE,
                            )
                            for li in range(u):
                                t = t0 + u0 + li
                                nc.tensor.matmul(
                                    attn_ps[:],
                                    lhsT=et[:, li * P:(li + 1) * P],
                                    rhs=kvt[:, w * P + (u0 + li) * VW:w * P + (u0 + li + 1) * VW],
                                    start=(t == 0), stop=(t == Tj - 1),
                                    skip_group_check=True,
                                )
                    recip = spool.tile([P, 1], dtf, tag="recip")
                    nc.vector.reciprocal(recip[:], attn_ps[:, D_V:D_V + 1])
                    nc.vector.tensor_scalar_mul(
                        attn_cc[:, j * D_V:(j + 1) * D_V], attn_ps[:, 0:D_V], recip[:],
                    )

            nc.scalar.dma_start(acc_in[:], attn_cc[:])
            nc.gpsimd.collective_compute(
                "AllToAll", mybir.AluOpType.bypass, replica_groups=rg,
                ins=[acc_in.opt()], outs=[acc_out.opt()],
            )

            # bm_all[(c,j), (hl,d)] = acc_out[c*16+hl, j*128+d]  (ACT ring)
            bm_all = cpool.tile([P, HPC * P], dtb)
            for cc in range(NC):
                nc.scalar.dma_start(
                    bm_all[cc * BPC:(cc + 1) * BPC, :]
                    .rearrange("j (hl d) -> j hl d", d=P),
                    acc_out.rearrange("(c hl) (j d) -> c j hl d", hl=HPC, d=P)[cc],
                )
            # rest of the wo stream (ring-paced by phase C consumption)
            for kt in range(22, 2 * HPC):
                half, k = divmod(kt, HPC)
                wt2 = wopoolA.tile([P, WO_HALF], dtb, tag="wt2")
                nc.gpsimd.dma_start(
                    wt2[:], wo_d.ap()[k * P:(k + 1) * P, half * WO_HALF:(half + 1) * WO_HALF])
                wo_tiles.append(wt2)

            # ---- phase C: Wo partial (this core's 16 heads, all 128 batches) ----
            with (
                tc.tile_pool(name="tps2", bufs=2, space="PSUM") as tps2,
                tc.tile_pool(name="wops", bufs=1, space="PSUM") as wops,
            ):
                lhs_tiles = []
                for hl in range(HPC):
                    lh_ps = tps2.tile([P, P], dtb, tag="tps2")
                    nc.tensor.transpose(lh_ps[:], bm_all[:, hl * P:(hl + 1) * P], idn[:])
                    lh = lhpool.tile([P, P], dtb, tag="lh")
                    nc.vector.tensor_copy(lh[:], lh_ps[:])
                    lhs_tiles.append(lh)
                for half in range(2):
                    wo_ps = wops.tile([P, WO_HALF], dtf, tag="wop")
                    for kt in range(HPC):
                        wt2 = wo_tiles[half * HPC + kt]
                        for c in range(WO_HALF // 512):
                            nc.tensor.matmul(
                                wo_ps[:, c * 512:(c + 1) * 512],
                                lhsT=lhs_tiles[kt][:],
                                rhs=wt2[:, c * 512:(c + 1) * 512],
                                start=(kt == 0), stop=(kt == HPC - 1),
                            )
                    out_sb = opool.tile([P, WO_HALF], dtb, tag="osb")
                    for cq in range(2):
                        c0 = cq * (WO_HALF // 2)
                        c1 = c0 + WO_HALF // 2
                        nc.vector.tensor_copy(out_sb[:, c0:c1], wo_ps[:, c0:c1])
                        nc.sync.dma_start(
                            out_d.ap()[:, half * WO_HALF + c0:half * WO_HALF + c1],
                            out_sb[:, c0:c1],
                        )
            wo_stack.close()   # close wopool before the outer pools (LIFO)

    if do_compile:
        nc.compile()
    return nc


def kernel(hidden_states, k_cache, v_cache, Wq, Wkv, Wo, positions, slot_mapping,
           seq_lens):
    global LAST_RESULTS
    h = np.asarray(hidden_states, np.float32)[:, -1, :]        # [B, HID]
    k_cache = np.asarray(k_cache, np.float32)
    v_cache = np.asarray(v_cache, np.float32)
    Wq = np.asarray(Wq, np.float32)
    Wkv = np.asarray(Wkv, np.float32)
    Wo = np.asarray(Wo, np.float32)
    seq = np.asarray(seq_lens).astype(np.int64)

    # ---- schedule: sort batches by length, slot j holds ranks [8j, 8j+8) ----
    order = np.argsort(-seq, kind="stable")
    batch_of = np.empty((NC, BPC), np.int64)
    for j in range(BPC):
        for c in range(NC):
            batch_of[c, j] = order[NC * j + c]
    perm = batch_of.reshape(-1)                                # sigma (c-major)
    L = np.array([int(seq[order[NC * j]]) for j in range(BPC)])
    T = (L + P - 1) // P

    # kv blob layout: per slot, groups of up to GRP tiles; group = kt|vv cols
    kvoff = []
    KVCOLS = 0
    for j in range(BPC):
        groups = _groups_of(int(T[j]), KVCOLS)
        KVCOLS = groups[-1][2] + groups[-1][1] * TW
        kvoff.append(groups)

    key = (KVCOLS, tuple(int(t) for t in T))
    if key not in _PROGRAM_CACHE:
        _PROGRAM_CACHE.clear()
        _PROGRAM_CACHE[key] = _build_program(T, kvoff, KVCOLS)
    nc = _PROGRAM_CACHE[key]

    # ---- host staging ----
    # h^T in device layout: [p, k*128 + b] = h_sigma[b, k*128+p]; staged /GQ
    # because wq is staged xGQ (fp8 subnormal headroom) -> q_ps at true scale.
    h_sigma = (h[perm] / GQ).astype(BF16)                      # [128, HID]
    ht_stage = np.ascontiguousarray(
        h_sigma.reshape(P, HID // P, P).transpose(2, 1, 0).reshape(P, HID)
    )
    idn_np = np.eye(P, dtype=BF16)

    in_maps = []
    for c in range(NC):
        cols = [Wq[:, (HPC * c + hl) * (D + D_ROPE):(HPC * c + hl) * (D + D_ROPE) + D]
                for hl in range(HPC)]
        wq_flat = np.concatenate(cols + [Wkv[:, :256]], axis=1) * GQ  # [5120, 2304]
        # pack 2 k-tiles per super-tile: col = s*2*QCOLS + half*QCOLS + q,
        # value = wq_flat[(2s+half)*128 + p, q]; first NK16 supertiles bf16,
        # remaining NK8 supertiles fp8-e3m4 (clipped to its +-15.5 range).
        wq_pack = (
            wq_flat.reshape(NK16 + NK8, 2, P, QCOLS)
            .transpose(2, 0, 1, 3)
            .reshape(P, (NK16 + NK8) * 2 * QCOLS)
        )
        wq8_stage = np.ascontiguousarray(
            np.clip(wq_pack, -15.0, 15.0)).astype(E3M4)
        wo_stage = np.ascontiguousarray(
            Wo[HPC * c * D_V:(HPC * c + HPC) * D_V, :]
        ).astype(BF16)

        kv_stage = np.zeros((P, KVCOLS), BF16)
        for j in range(BPC):
            b = int(batch_of[c, j])
            S = int(seq[b])
            Tj = int(T[j])
            # kt block [128 d, Tj*128 s]: position 0 = new token (patched on
            # device), positions 1..S-1 = cache rows 0..S-2, rest 0.
            ktb = np.zeros((P, Tj * P), BF16)
            ktb[:, 1:S] = k_cache[b, :S - 1, :].T.astype(BF16)
            # vv block tiles [128 s, 129]: col 128 = valid mask
            vblk = np.zeros((Tj * P, VW), BF16)
            vblk[1:S, :D_V] = v_cache[b, :S - 1, :].astype(BF16)
            vblk[:S, D_V] = 1.0
            vtiles = vblk.reshape(Tj, P, VW)
            for (t0, w, off) in kvoff[j]:
                kw = w * P
                kv_stage[:, off:off + kw] = ktb[:, t0 * P:(t0 + w) * P]
                kv_stage[:, off + kw:off + kw + w * VW] = (
                    vtiles[t0:t0 + w].transpose(1, 0, 2).reshape(P, w * VW)
                )
        in_maps.append({
            "ht": ht_stage, "wq8": wq8_stage,
            "kv": kv_stage, "wo": wo_stage, "idn": idn_np,
        })

    trace = os.environ.get("BASS_KERNEL_TRACE", "0") == "1"
    if trace:
        _install_ntff_hook()
    kw = {}
    tcs = os.environ.get("BASS_TRACE_CORES", "")
    if trace and tcs:
        kw["trace_cores"] = list(range(int(tcs)))
    res = run_bass_kernel_spmd(nc, in_maps, core_ids=list(range(NC)), trace=trace, **kw)
    LAST_RESULTS = res

    out_sigma = np.zeros((P, HID), np.float32)
    for c in range(NC):
        out_sigma += res.results[c]["outp"].astype(np.float32)
    out_full = np.empty((B, HID), np.float32)
    out_full[perm] = out_sigma
    return out_full.reshape(B, 1, HID)


# revision 34
# speedup vs baseline: 1.0368x; 1.0368x over previous
"""DeepSeekV2-style MLA decode attention (MQA, B=128 decode tokens) on 8 trn2 NeuronCores.

v10 (verified 271145 ns, rel err 0.018808): head-sharded Wq/Wo + batch-sharded
attention with two AllToAlls. DMA priority enforced via real data deps (Tile
schedules by deps, not program order): kv ring gated behind the last wq tile's
DMA, wo prefetch gated behind the kv prefetch and the qj gathers, all via
dummy pool-buffer writes. Wq in fp8-e3m4 (x64 scale, h staged /64, clip +-15).
Warm-up AllToAll absorbs the first-collective Mesh setup. wq/hts pools scoped
to phase A so the 32-tile wo stream reuses their SBUF. kv cache as one
host-staged blob ([kt | vv+mask] 16-tile groups, ~1MB DMAs); mask column
doubles as the softmax denominator via the PV matmul.
"""

import os
import numpy as np
import ml_dtypes

import concourse.bass as bass
import concourse.mybir as mybir
import concourse.tile as tile
from concourse import bacc
from concourse.bass_utils import run_bass_kernel_spmd

BF16 = ml_dtypes.bfloat16
P = 128
B, MAX_S, HID = 128, 4096, 5120
H, D, D_ROPE, D_V = 128, 128, 64, 128
NC, HPC, BPC = 8, 16, 16           # cores, heads/core, slots(batches)/core
QCOLS = HPC * D + 256              # 2304: per-core q cols + kv cols
SCALE = float(D) ** -0.5
PROJ_CHUNKS = [(0, 512), (512, 512), (1024, 512), (1536, 512), (2048, 256)]
VW = D_V + 1                       # 129: v tile width incl. mask column
TW = P + VW                        # 257: cols per s-tile in the kv blob
GRP = 16                           # s-tiles per kv DMA group (~1MB transfers)
SUB = 4                            # s-tiles per exp/psum sub-chunk
NK16 = 0                           # bf16 wq super-tiles (2 k-tiles each)
NK8 = 20                           # fp8-e3m4 wq super-tiles (2 k-tiles each)
GQ = 64.0                          # wq stage scale (h staged /GQ)
WO_HALF = HID // 2                 # 2560
KV_PRE = 6                         # kv ring depth (groups)
WO_PRE = 22                        # wo ring depth (tiles; reuses wq space)

dtb = mybir.dt.bfloat16
dtf = mybir.dt.float32
dt8 = mybir.dt.float8e3
E3M4 = ml_dtypes.float8_e3m4

_PROGRAM_CACHE: dict = {}
LAST_RESULTS = None


def _install_ntff_hook():
    import sys
    import types
    try:
        import antenv.axon_hooks  # noqa: F401
        return
    except ImportError:
        pass
    try:
        from trn_agent_boot.trn_boot import _ntff_profile_via_ctypes
        hook = _ntff_profile_via_ctypes("/opt/axon/libaxon_pjrt.so")
        mod = types.ModuleType("antenv.axon_hooks")
        mod._hook = hook
        mod.set_axon_ntff_profile_hook = lambda h: setattr(mod, "_hook", h)
        mod.get_axon_ntff_profile_hook = lambda: mod._hook
        sys.modules["antenv.axon_hooks"] = mod
        import antenv
        antenv.axon_hooks = mod
    except Exception:
        pass


def _groups_of(Tj, off0):
    """[(t0, w, off), ...] groups of up to GRP s-tiles; off = blob col offset."""
    out = []
    t, off = 0, off0
    while t < Tj:
        w = min(GRP, Tj - t)
        out.append((t, w, off))
        off += w * TW
        t += w
    return out


def _build_program(T, kvoff, KVCOLS, do_compile=True):
    """Build + compile the per-core bass program. T: per-slot tile counts;
    kvoff[j]: [(t0, w, off)] groups of slot j in the kv blob."""
    nc = bacc.Bacc("TRN2", target_bir_lowering=False, debug=False, num_devices=NC)

    ht_d = nc.dram_tensor("ht", [P, HID], dtb, kind="ExternalInput")
    wq8_d = nc.dram_tensor("wq8", [P, NK8 * 2 * QCOLS], dt8, kind="ExternalInput")
    kv_d = nc.dram_tensor("kv", [P, KVCOLS], dtb, kind="ExternalInput")
    wo_d = nc.dram_tensor("wo", [HPC * D_V, HID], dtb, kind="ExternalInput")
    idn_d = nc.dram_tensor("idn", [P, P], dtb, kind="ExternalInput")
    out_d = nc.dram_tensor("outp", [P, HID], dtb, kind="ExternalOutput")

    rg = [list(range(NC))]
    Exp = mybir.ActivationFunctionType.Exp

    from contextlib import ExitStack
    wo_stack = ExitStack()
    with tile.TileContext(nc) as tc, wo_stack:
        with (
            tc.tile_pool(name="cpool", bufs=1) as cpool,
            tc.tile_pool(name="kvpool", bufs=KV_PRE) as kvpool,
            tc.tile_pool(name="qtpool", bufs=3) as qtpool,
            tc.tile_pool(name="epool", bufs=4) as epool,
            tc.tile_pool(name="spool", bufs=2) as spool,
            tc.tile_pool(name="lhpool", bufs=HPC) as lhpool,
            tc.tile_pool(name="opool", bufs=2) as opool,
            tc.tile_pool(name="dram", bufs=1, space="DRAM") as dram,
        ):
            # ---- constants / global loads (sync ring: first) ----
            idn = cpool.tile([P, P], dtb)
            nc.sync.dma_start(idn[:], idn_d.ap())

            qcc_in = dram.tile([P, QCOLS], dtb)
            qcc_out = dram.tile([P, QCOLS], dtb)
            acc_in = dram.tile([P, BPC * D_V], dtb)
            acc_out = dram.tile([P, BPC * D_V], dtb)

            # tiny warm-up AllToAll: absorbs the first-collective Mesh setup
            # (~11.5us trigger delay) into the idle preamble window.
            wrm_in = dram.tile([P, 16], dtb)
            wrm_out = dram.tile([P, 16], dtb)
            nc.gpsimd.collective_compute(
                "AllToAll", mybir.AluOpType.bypass, replica_groups=rg,
                ins=[wrm_in.opt()], outs=[wrm_out.opt()],
            )

            # ---- phase A: q/kv projection (this core's 16 heads, all 128 batches)
            # all 40 k-tiles fp8-e3m4; accumulate into one psum.
            # wq pools + hts live in a scope that closes after phase A so the
            # wo pool below can reuse their SBUF (Tile adds the WAR sync).
            qc_sb = cpool.tile([P, QCOLS], dtb)
            with (
                tc.tile_pool(name="wq8pool", bufs=6) as wq8pool,
                tc.tile_pool(name="htpool", bufs=1) as htpool,
            ):
                hts = htpool.tile([P, HID], dtb)
                nc.sync.dma_start(hts[:], ht_d.ap())
                wt_last = None
                with tc.tile_pool(name="pjp", bufs=1, space="PSUM") as pjp:
                    q_ps = pjp.tile([P, QCOLS], dtf)
                    for s in range(NK16 + NK8):
                        wt = wq8pool.tile([P, 2 * QCOLS], dt8, tag="wt8")
                        nc.sync.dma_start(
                            wt[:], wq8_d.ap()[:, s * 2 * QCOLS:(s + 1) * 2 * QCOLS])
                        wt_last = wt
                        for half in range(2):
                            k = 2 * s + half
                            for (c0, cw) in PROJ_CHUNKS:
                                nc.tensor.matmul(
                                    q_ps[:, c0:c0 + cw],
                                    lhsT=hts[:, k * P:(k + 1) * P],
                                    rhs=wt[:, half * QCOLS + c0:half * QCOLS + c0 + cw],
                                    start=(k == 0),
                                    stop=(k == 2 * (NK16 + NK8) - 1),
                                )
                    # copy per chunk so the tail overlaps the last matmuls
                    for (c0, cw) in PROJ_CHUNKS:
                        nc.vector.tensor_copy(qc_sb[:, c0:c0 + cw], q_ps[:, c0:c0 + cw])

                # kv ring gated behind the last wq DMA: Tile schedules by data
                # deps, so priority is enforced by writing a dummy byte (dep on
                # wq's last tile DMA) into each ring buffer before its first
                # real load. Emitted inside this scope (reads wt_last).
                for _ in range(KV_PRE):
                    dmy = kvpool.tile([P, GRP * TW], dtb, tag="kv")
                    nc.vector.tensor_copy(dmy[0:1, 0:1], wt_last[0:1, 0:1])

            # ---- kv blob loads (sync ring) ----
            kv_tiles = []
            for j in range(BPC):
                for (t0, w, off) in kvoff[j]:
                    kvt = kvpool.tile([P, GRP * TW], dtb, tag="kv")
                    nc.sync.dma_start(kvt[:, 0:w * TW], kv_d.ap()[:, off:off + w * TW])
                    kv_tiles.append(kvt)

            # ---- A2A1 staging (ACT ring) + wo prefetch (gpsimd ring) ----
            nc.scalar.dma_start(qcc_in[:], qc_sb[:])
            # wo tiles 0-13 (pool A) gated behind the kv prefetch; tiles
            # 14-21 (pool B, emitted after the qj gathers) gated behind those
            # gathers so they don't starve phase B's start; tiles 22-31 reuse
            # pool A buffers (paced by phase C consumption).
            wopoolA = wo_stack.enter_context(tc.tile_pool(name="wopoolA", bufs=14))
            wopoolB = wo_stack.enter_context(tc.tile_pool(name="wopoolB", bufs=8))
            for i in range(14):
                dmy = wopoolA.tile([P, WO_HALF], dtb, tag="wt2")
                nc.vector.tensor_copy(dmy[0:1, 0:1], kv_tiles[KV_PRE - 1][0:1, 0:1])
            wo_tiles = []
            for kt in range(14):
                half, k = divmod(kt, HPC)
                wt2 = wopoolA.tile([P, WO_HALF], dtb, tag="wt2")
                nc.gpsimd.dma_start(
                    wt2[:], wo_d.ap()[k * P:(k + 1) * P, half * WO_HALF:(half + 1) * WO_HALF])
                wo_tiles.append(wt2)

            nc.gpsimd.collective_compute(
                "AllToAll", mybir.AluOpType.bypass, replica_groups=rg,
                ins=[qcc_in.opt()], outs=[qcc_out.opt()],
            )

            # ---- phase B gathers (ACT ring: idle at this point) ----
            # qj_all[(c,hl), (j,d)] = qcc_out[c*16+j, hl*128+d]
            qj_all = cpool.tile([P, BPC * P], dtb)
            for cc in range(NC):
                nc.scalar.dma_start(
                    qj_all[cc * HPC:(cc + 1) * HPC, :]
                    .rearrange("hl (j d) -> hl j d", d=P),
                    qcc_out.rearrange("(c j) (hl d) -> c hl j d", j=BPC, d=P)[cc, 0:HPC],
                )
            knew = cpool.tile([BPC, P], dtb)
            nc.scalar.dma_start(knew[:], qcc_out[0:BPC, HPC * D:HPC * D + P])
            vnew = cpool.tile([1, BPC * P], dtb)
            nc.scalar.dma_start(
                vnew.rearrange("one (j d) -> one j d", d=P),
                qcc_out.rearrange("(c j) f -> c j f", j=BPC)[0:1, :, HPC * D + P:QCOLS],
            )

            # ---- phase B: attention over this core's 16 batches ----
            attn_cc = cpool.tile([P, BPC * D_V], dtb)
            with (
                tc.tile_pool(name="tps", bufs=2, space="PSUM") as tps,
                tc.tile_pool(name="scps", bufs=3, space="PSUM") as scps,
                tc.tile_pool(name="aps", bufs=2, space="PSUM") as aps,
            ):
                knewT_ps = tps.tile([P, BPC], dtb, tag="tps")
                nc.tensor.transpose(knewT_ps[:], knew[:], idn[0:BPC, 0:BPC])
                knewT = cpool.tile([P, BPC], dtb)
                nc.vector.tensor_copy(knewT[:], knewT_ps[:])

                gi = 0
                for j in range(BPC):
                    Tj = int(T[j])
                    qjT_ps = tps.tile([P, P], dtb, tag="tps")
                    nc.tensor.transpose(qjT_ps[:], qj_all[:, j * P:(j + 1) * P], idn[:])
                    qjT = qtpool.tile([P, P], dtb, tag="qjT")
                    nc.vector.tensor_copy(qjT[:], qjT_ps[:])

                    attn_ps = aps.tile([P, VW], dtf, tag="attn")
                    for (t0, w, off) in kvoff[j]:
                        kvt = kv_tiles[gi]
                        gi += 1
                        if t0 == 0:
                            # patch new-token k (column 0 of kt) and v (row 0
                            # of vv values; its mask col is host-staged 1.0)
                            nc.vector.tensor_copy(kvt[:, 0:1], knewT[:, j:j + 1])
                            nc.vector.tensor_copy(
                                kvt[0:1, w * P:w * P + D_V],
                                vnew[0:1, j * P:j * P + D_V],
                            )
                        for u0 in range(0, w, SUB):
                            u = min(SUB, w - u0)
                            sc = scps.tile([P, SUB * P], dtf, tag="sc")
                            for li in range(u):
                                nc.tensor.matmul(
                                    sc[:, li * P:(li + 1) * P],
                                    lhsT=kvt[:, (u0 + li) * P:(u0 + li + 1) * P],
                                    rhs=qjT[:],
                                    start=True, stop=True,
                                )
                            et = epool.tile([P, SUB * P], dtb, tag="et")
                            nc.scalar.activation(
                                et[:, 0:u * P], sc[:, 0:u * P], Exp,
                                bias=0.0, scale=SCALE,
                            )
                            for li in range(u):
                                t = t0 + u0 + li
                                nc.tensor.matmul(
                                    attn_ps[:],
                                    lhsT=et[:, li * P:(li + 1) * P],
                                    rhs=kvt[:, w * P + (u0 + li) * VW:w * P + (u0 + li + 1) * VW],
                                    start=(t == 0), stop=(t == Tj - 1),
                                    skip_group_check=True,
                                )
                    recip = spool.tile([P, 1], dtf, tag="recip")
                    nc.vector.reciprocal(recip[:], attn_ps[:, D_V:D_V + 1])
                    nc.vector.tensor_scalar_mul(
                        attn_cc[:, j * D_V:(j + 1) * D_V], attn_ps[:, 0:D_V], recip[:],
                    )

            # wo tiles 14-21: gated on phase B completion (last slot's attn),
            # so their 5.2MB stream fills the otherwise-idle A2A2 window and
            # the phase-B kv stream gets the full HBM bandwidth.
            for i in range(8):
                dmy = wopoolB.tile([P, WO_HALF], dtb, tag="wt3")
                nc.vector.tensor_copy(
                    dmy[:, 0:1], attn_cc[:, (BPC - 1) * D_V:(BPC - 1) * D_V + 1])
            for kt in range(14, 22):
                half, k = divmod(kt, HPC)
                wt2 = wopoolB.tile([P, WO_HALF], dtb, tag="wt3")
                nc.gpsimd.dma_start(
                    wt2[:], wo_d.ap()[k * P:(k + 1) * P, half * WO_HALF:(half + 1) * WO_HALF])
                wo_tiles.append(wt2)

            nc.scalar.dma_start(acc_in[:], attn_cc[:])
            nc.gpsimd.collective_compute(
                "AllToAll", mybir.AluOpType.bypass, replica_groups=rg,
                ins=[acc_in.opt()], outs=[acc_out.opt()],
            )

            # bm_all[(c,j), (hl,d)] = acc_out[c*16+hl, j*128+d]  (ACT ring)
            bm_all = cpool.tile([P, HPC * P], dtb)
            for cc in range(NC):
                nc.scalar.dma_start(
                    bm_all[cc * BPC:(cc + 1) * BPC, :]
                    .rearrange("j (hl d) -> j hl d", d=P),
                    acc_out.rearrange("(c hl) (j d) -> c j hl d", hl=HPC, d=P)[cc],
                )
            # rest of the wo stream (ring-paced by phase C consumption)
            for kt in range(22, 2 * HPC):
                half, k = divmod(kt, HPC)
                wt2 = wopoolA.tile([P, WO_HALF], dtb, tag="wt2")
                nc.gpsimd.dma_start(
                    wt2[:], wo_d.ap()[k * P:(k + 1) * P, half * WO_HALF:(half + 1) * WO_HALF])
                wo_tiles.append(wt2)

            # ---- phase C: Wo partial (this core's 16 heads, all 128 batches) ----
            with (
                tc.tile_pool(name="tps2", bufs=2, space="PSUM") as tps2,
                tc.tile_pool(name="wops", bufs=1, space="PSUM") as wops,
            ):
                lhs_tiles = []
                for hl in range(HPC):
                    lh_ps = tps2.tile([P, P], dtb, tag="tps2")
                    nc.tensor.transpose(lh_ps[:], bm_all[:, hl * P:(hl + 1) * P], idn[:])
                    lh = lhpool.tile([P, P], dtb, tag="lh")
                    nc.vector.tensor_copy(lh[:], lh_ps[:])
                    lhs_tiles.append(lh)
                for half in range(2):
                    wo_ps = wops.tile([P, WO_HALF], dtf, tag="wop")
                    for kt in range(HPC):
                        wt2 = wo_tiles[half * HPC + kt]
                        for c in range(WO_HALF // 512):
                            nc.tensor.matmul(
                                wo_ps[:, c * 512:(c + 1) * 512],
                                lhsT=lhs_tiles[kt][:],
                                rhs=wt2[:, c * 512:(c + 1) * 512],
                                start=(kt == 0), stop=(kt == HPC - 1),
                            )
                    out_sb = opool.tile([P, WO_HALF], dtb, tag="osb")
                    nc.vector.tensor_copy(out_sb[:], wo_ps[:])
                    nc.sync.dma_start(
                        out_d.ap()[:, half * WO_HALF:(half + 1) * WO_HALF], out_sb[:],
                    )
            wo_stack.close()   # close wopool before the outer pools (LIFO)

    if do_compile:
        nc.compile()
    return nc


def kernel(hidden_states, k_cache, v_cache, Wq, Wkv, Wo, positions, slot_mapping,
           seq_lens):
    global LAST_RESULTS
    h = np.asarray(hidden_states, np.float32)[:, -1, :]        # [B, HID]
    k_cache = np.asarray(k_cache, np.float32)
    v_cache = np.asarray(v_cache, np.float32)
    Wq = np.asarray(Wq, np.float32)
    Wkv = np.asarray(Wkv, np.float32)
    Wo = np.asarray(Wo, np.float32)
    seq = np.asarray(seq_lens).astype(np.int64)

    # ---- schedule: sort batches by length, slot j holds ranks [8j, 8j+8) ----
    order = np.argsort(-seq, kind="stable")
    batch_of = np.empty((NC, BPC), np.int64)
    for j in range(BPC):
        for c in range(NC):
            batch_of[c, j] = order[NC * j + c]
    perm = batch_of.reshape(-1)                                # sigma (c-major)
    L = np.array([int(seq[order[NC * j]]) for j in range(BPC)])
    T = (L + P - 1) // P

    # kv blob layout: per slot, groups of up to GRP tiles; group = kt|vv cols
    kvoff = []
    KVCOLS = 0
    for j in range(BPC):
        groups = _groups_of(int(T[j]), KVCOLS)
        KVCOLS = groups[-1][2] + groups[-1][1] * TW
        kvoff.append(groups)

    key = (KVCOLS, tuple(int(t) for t in T))
    if key not in _PROGRAM_CACHE:
        _PROGRAM_CACHE.clear()
        _PROGRAM_CACHE[key] = _build_program(T, kvoff, KVCOLS)
    nc = _PROGRAM_CACHE[key]

    # ---- host staging ----
    # h^T in device layout: [p, k*128 + b] = h_sigma[b, k*128+p]; staged /GQ
    # because wq is staged xGQ (fp8 subnormal headroom) -> q_ps at true scale.
    h_sigma = (h[perm] / GQ).astype(BF16)                      # [128, HID]
    ht_stage = np.ascontiguousarray(
        h_sigma.reshape(P, HID // P, P).transpose(2, 1, 0).reshape(P, HID)
    )
    idn_np = np.eye(P, dtype=BF16)

    in_maps = []
    for c in range(NC):
        cols = [Wq[:, (HPC * c + hl) * (D + D_ROPE):(HPC * c + hl) * (D + D_ROPE) + D]
                for hl in range(HPC)]
        wq_flat = np.concatenate(cols + [Wkv[:, :256]], axis=1) * GQ  # [5120, 2304]
        # pack 2 k-tiles per super-tile: col = s*2*QCOLS + half*QCOLS + q,
        # value = wq_flat[(2s+half)*128 + p, q]; first NK16 supertiles bf16,
        # remaining NK8 supertiles fp8-e3m4 (clipped to its +-15.5 range).
        wq_pack = (
            wq_flat.reshape(NK16 + NK8, 2, P, QCOLS)
            .transpose(2, 0, 1, 3)
            .reshape(P, (NK16 + NK8) * 2 * QCOLS)
        )
        wq8_stage = np.ascontiguousarray(
            np.clip(wq_pack, -15.0, 15.0)).astype(E3M4)
        wo_stage = np.ascontiguousarray(
            Wo[HPC * c * D_V:(HPC * c + HPC) * D_V, :]
        ).astype(BF16)

        kv_stage = np.zeros((P, KVCOLS), BF16)
        for j in range(BPC):
            b = int(batch_of[c, j])
            S = int(seq[b])
            Tj = int(T[j])
            # kt block [128 d, Tj*128 s]: position 0 = new token (patched on
            # device), positions 1..S-1 = cache rows 0..S-2, rest 0.
            ktb = np.zeros((P, Tj * P), BF16)
            ktb[:, 1:S] = k_cache[b, :S - 1, :].T.astype(BF16)
            # vv block tiles [128 s, 129]: col 128 = valid mask
            vblk = np.zeros((Tj * P, VW), BF16)
            vblk[1:S, :D_V] = v_cache[b, :S - 1, :].astype(BF16)
            vblk[:S, D_V] = 1.0
            vtiles = vblk.reshape(Tj, P, VW)
            for (t0, w, off) in kvoff[j]:
                kw = w * P
                kv_stage[:, off:off + kw] = ktb[:, t0 * P:(t0 + w) * P]
                kv_stage[:, off + kw:off + kw + w * VW] = (
                    vtiles[t0:t0 + w].transpose(1, 0, 2).reshape(P, w * VW)
                )
        in_maps.append({
            "ht": ht_stage, "wq8": wq8_stage,
            "kv": kv_stage, "wo": wo_stage, "idn": idn_np,
        })

    trace = os.environ.get("BASS_KERNEL_TRACE", "0") == "1"
    if trace:
        _install_ntff_hook()
    kw = {}
    tcs = os.environ.get("BASS_TRACE_CORES", "")
    if trace and tcs:
        kw["trace_cores"] = list(range(int(tcs)))
    res = run_bass_kernel_spmd(nc, in_maps, core_ids=list(range(NC)), trace=trace, **kw)
    LAST_RESULTS = res

    out_sigma = np.zeros((P, HID), np.float32)
    for c in range(NC):
        out_sigma += res.results[c]["outp"].astype(np.float32)
    out_full = np.empty((B, HID), np.float32)
    out_full[perm] = out_sigma
    return out_full.reshape(B, 1, HID)


# revision 35
# speedup vs baseline: 1.0412x; 1.0042x over previous
"""DeepSeekV2-style MLA decode attention (MQA, B=128 decode tokens) on 8 trn2 NeuronCores.

v10 (verified 271145 ns, rel err 0.018808): head-sharded Wq/Wo + batch-sharded
attention with two AllToAlls. DMA priority enforced via real data deps (Tile
schedules by deps, not program order): kv ring gated behind the last wq tile's
DMA, wo prefetch gated behind the kv prefetch and the qj gathers, all via
dummy pool-buffer writes. Wq in fp8-e3m4 (x64 scale, h staged /64, clip +-15).
Warm-up AllToAll absorbs the first-collective Mesh setup. wq/hts pools scoped
to phase A so the 32-tile wo stream reuses their SBUF. kv cache as one
host-staged blob ([kt | vv+mask] 16-tile groups, ~1MB DMAs); mask column
doubles as the softmax denominator via the PV matmul.
"""

import os
import numpy as np
import ml_dtypes

import concourse.bass as bass
import concourse.mybir as mybir
import concourse.tile as tile
from concourse import bacc
from concourse.bass_utils import run_bass_kernel_spmd

BF16 = ml_dtypes.bfloat16
P = 128
B, MAX_S, HID = 128, 4096, 5120
H, D, D_ROPE, D_V = 128, 128, 64, 128
NC, HPC, BPC = 8, 16, 16           # cores, heads/core, slots(batches)/core
QCOLS = HPC * D + 256              # 2304: per-core q cols + kv cols
SCALE = float(D) ** -0.5
PROJ_CHUNKS = [(0, 512), (512, 512), (1024, 512), (1536, 512), (2048, 256)]
VW = D_V + 1                       # 129: v tile width incl. mask column
TW = P + VW                        # 257: cols per s-tile in the kv blob
GRP = 16                           # s-tiles per kv DMA group (~1MB transfers)
SUB = 4                            # s-tiles per exp/psum sub-chunk
NK16 = 0                           # bf16 wq super-tiles (2 k-tiles each)
NK8 = 20                           # fp8-e3m4 wq super-tiles (2 k-tiles each)
GQ = 64.0                          # wq stage scale (h staged /GQ)
WO_HALF = HID // 2                 # 2560
KV_PRE = 6                         # kv ring depth (groups)
WO_PRE = 22                        # wo ring depth (tiles; reuses wq space)

dtb = mybir.dt.bfloat16
dtf = mybir.dt.float32
dt8 = mybir.dt.float8e3
E3M4 = ml_dtypes.float8_e3m4

_PROGRAM_CACHE: dict = {}
LAST_RESULTS = None


def _install_ntff_hook():
    import sys
    import types
    try:
        import antenv.axon_hooks  # noqa: F401
        return
    except ImportError:
        pass
    try:
        from trn_agent_boot.trn_boot import _ntff_profile_via_ctypes
        hook = _ntff_profile_via_ctypes("/opt/axon/libaxon_pjrt.so")
        mod = types.ModuleType("antenv.axon_hooks")
        mod._hook = hook
        mod.set_axon_ntff_profile_hook = lambda h: setattr(mod, "_hook", h)
        mod.get_axon_ntff_profile_hook = lambda: mod._hook
        sys.modules["antenv.axon_hooks"] = mod
        import antenv
        antenv.axon_hooks = mod
    except Exception:
        pass


def _groups_of(Tj, off0):
    """[(t0, w, off), ...] groups of up to GRP s-tiles; off = blob col offset."""
    out = []
    t, off = 0, off0
    while t < Tj:
        w = min(GRP, Tj - t)
        out.append((t, w, off))
        off += w * TW
        t += w
    return out


def _build_program(T, kvoff, KVCOLS, do_compile=True):
    """Build + compile the per-core bass program. T: per-slot tile counts;
    kvoff[j]: [(t0, w, off)] groups of slot j in the kv blob."""
    nc = bacc.Bacc("TRN2", target_bir_lowering=False, debug=False, num_devices=NC)

    ht_d = nc.dram_tensor("ht", [P, HID], dtb, kind="ExternalInput")
    wq8_d = nc.dram_tensor("wq8", [P, NK8 * 2 * QCOLS], dt8, kind="ExternalInput")
    kv_d = nc.dram_tensor("kv", [P, KVCOLS], dtb, kind="ExternalInput")
    wo_d = nc.dram_tensor("wo", [HPC * D_V, HID], dtb, kind="ExternalInput")
    idn_d = nc.dram_tensor("idn", [P, P], dtb, kind="ExternalInput")
    out_d = nc.dram_tensor("outp", [P, HID], dtb, kind="ExternalOutput")

    rg = [list(range(NC))]
    Exp = mybir.ActivationFunctionType.Exp

    from contextlib import ExitStack
    wo_stack = ExitStack()
    with tile.TileContext(nc) as tc, wo_stack:
        with (
            tc.tile_pool(name="cpool", bufs=1) as cpool,
            tc.tile_pool(name="kvpool", bufs=KV_PRE) as kvpool,
            tc.tile_pool(name="qtpool", bufs=3) as qtpool,
            tc.tile_pool(name="epool", bufs=4) as epool,
            tc.tile_pool(name="spool", bufs=2) as spool,
            tc.tile_pool(name="lhpool", bufs=HPC) as lhpool,
            tc.tile_pool(name="opool", bufs=2) as opool,
            tc.tile_pool(name="dram", bufs=1, space="DRAM") as dram,
        ):
            # ---- constants / global loads (sync ring: first) ----
            idn = cpool.tile([P, P], dtb)
            nc.sync.dma_start(idn[:], idn_d.ap())

            qcc_in = dram.tile([P, QCOLS], dtb)
            qcc_out = dram.tile([P, QCOLS], dtb)
            acc_in = dram.tile([P, BPC * D_V], dtb)
            acc_out = dram.tile([P, BPC * D_V], dtb)

            # tiny warm-up AllToAll: absorbs the first-collective Mesh setup
            # (~11.5us trigger delay) into the idle preamble window.
            wrm_in = dram.tile([P, 16], dtb)
            wrm_out = dram.tile([P, 16], dtb)
            nc.gpsimd.collective_compute(
                "AllToAll", mybir.AluOpType.bypass, replica_groups=rg,
                ins=[wrm_in.opt()], outs=[wrm_out.opt()],
            )

            # ---- phase A: q/kv projection (this core's 16 heads, all 128 batches)
            # all 40 k-tiles fp8-e3m4; accumulate into one psum.
            # wq pools + hts live in a scope that closes after phase A so the
            # wo pool below can reuse their SBUF (Tile adds the WAR sync).
            qc_sb = cpool.tile([P, QCOLS], dtb)
            with (
                tc.tile_pool(name="wq8pool", bufs=6) as wq8pool,
                tc.tile_pool(name="htpool", bufs=1) as htpool,
            ):
                hts = htpool.tile([P, HID], dtb)
                nc.sync.dma_start(hts[:], ht_d.ap())
                wt_last = None
                with tc.tile_pool(name="pjp", bufs=1, space="PSUM") as pjp:
                    q_ps = pjp.tile([P, QCOLS], dtf)
                    for s in range(NK16 + NK8):
                        wt = wq8pool.tile([P, 2 * QCOLS], dt8, tag="wt8")
                        nc.sync.dma_start(
                            wt[:], wq8_d.ap()[:, s * 2 * QCOLS:(s + 1) * 2 * QCOLS])
                        wt_last = wt
                        for half in range(2):
                            k = 2 * s + half
                            for (c0, cw) in PROJ_CHUNKS:
                                nc.tensor.matmul(
                                    q_ps[:, c0:c0 + cw],
                                    lhsT=hts[:, k * P:(k + 1) * P],
                                    rhs=wt[:, half * QCOLS + c0:half * QCOLS + c0 + cw],
                                    start=(k == 0),
                                    stop=(k == 2 * (NK16 + NK8) - 1),
                                )
                    # copy per chunk so the tail overlaps the last matmuls
                    for (c0, cw) in PROJ_CHUNKS:
                        nc.vector.tensor_copy(qc_sb[:, c0:c0 + cw], q_ps[:, c0:c0 + cw])

                # kv ring gated behind the last wq DMA: Tile schedules by data
                # deps, so priority is enforced by writing a dummy byte (dep on
                # wq's last tile DMA) into each ring buffer before its first
                # real load. Emitted inside this scope (reads wt_last).
                for _ in range(KV_PRE):
                    dmy = kvpool.tile([P, GRP * TW], dtb, tag="kv")
                    nc.vector.tensor_copy(dmy[0:1, 0:1], wt_last[0:1, 0:1])

            # ---- kv blob loads (sync ring) ----
            kv_tiles = []
            for j in range(BPC):
                for (t0, w, off) in kvoff[j]:
                    kvt = kvpool.tile([P, GRP * TW], dtb, tag="kv")
                    nc.sync.dma_start(kvt[:, 0:w * TW], kv_d.ap()[:, off:off + w * TW])
                    kv_tiles.append(kvt)

            # ---- A2A1 staging (ACT ring) + wo prefetch (gpsimd ring) ----
            nc.scalar.dma_start(qcc_in[:], qc_sb[:])
            # wo tiles 0-13 (pool A) gated behind the kv prefetch; tiles
            # 14-21 (pool B, emitted after the qj gathers) gated behind those
            # gathers so they don't starve phase B's start; tiles 22-31 reuse
            # pool A buffers (paced by phase C consumption).
            wopoolA = wo_stack.enter_context(tc.tile_pool(name="wopoolA", bufs=14))
            wopoolB = wo_stack.enter_context(tc.tile_pool(name="wopoolB", bufs=8))
            for i in range(14):
                dmy = wopoolA.tile([P, WO_HALF], dtb, tag="wt2")
                nc.vector.tensor_copy(dmy[0:1, 0:1], kv_tiles[KV_PRE - 1][0:1, 0:1])
            wo_tiles = []
            for kt in range(14):
                half, k = divmod(kt, HPC)
                wt2 = wopoolA.tile([P, WO_HALF], dtb, tag="wt2")
                nc.gpsimd.dma_start(
                    wt2[:], wo_d.ap()[k * P:(k + 1) * P, half * WO_HALF:(half + 1) * WO_HALF])
                wo_tiles.append(wt2)

            nc.gpsimd.collective_compute(
                "AllToAll", mybir.AluOpType.bypass, replica_groups=rg,
                ins=[qcc_in.opt()], outs=[qcc_out.opt()],
            )

            # ---- phase B gathers (ACT ring: idle at this point) ----
            # qj_all[(c,hl), (j,d)] = qcc_out[c*16+j, hl*128+d]
            qj_all = cpool.tile([P, BPC * P], dtb)
            for cc in range(NC):
                nc.scalar.dma_start(
                    qj_all[cc * HPC:(cc + 1) * HPC, :]
                    .rearrange("hl (j d) -> hl j d", d=P),
                    qcc_out.rearrange("(c j) (hl d) -> c hl j d", j=BPC, d=P)[cc, 0:HPC],
                )
            knew = cpool.tile([BPC, P], dtb)
            nc.scalar.dma_start(knew[:], qcc_out[0:BPC, HPC * D:HPC * D + P])
            vnew = cpool.tile([1, BPC * P], dtb)
            nc.scalar.dma_start(
                vnew.rearrange("one (j d) -> one j d", d=P),
                qcc_out.rearrange("(c j) f -> c j f", j=BPC)[0:1, :, HPC * D + P:QCOLS],
            )

            # ---- phase B: attention over this core's 16 batches ----
            attn_cc = cpool.tile([P, BPC * D_V], dtb)
            with (
                tc.tile_pool(name="tps", bufs=2, space="PSUM") as tps,
                tc.tile_pool(name="scps", bufs=3, space="PSUM") as scps,
                tc.tile_pool(name="aps", bufs=2, space="PSUM") as aps,
            ):
                knewT_ps = tps.tile([P, BPC], dtb, tag="tps")
                nc.tensor.transpose(knewT_ps[:], knew[:], idn[0:BPC, 0:BPC])
                knewT = cpool.tile([P, BPC], dtb)
                nc.vector.tensor_copy(knewT[:], knewT_ps[:])

                gi = 0
                for j in range(BPC):
                    Tj = int(T[j])
                    qjT_ps = tps.tile([P, P], dtb, tag="tps")
                    nc.tensor.transpose(qjT_ps[:], qj_all[:, j * P:(j + 1) * P], idn[:])
                    qjT = qtpool.tile([P, P], dtb, tag="qjT")
                    nc.vector.tensor_copy(qjT[:], qjT_ps[:])

                    attn_ps = aps.tile([P, VW], dtf, tag="attn")
                    for (t0, w, off) in kvoff[j]:
                        kvt = kv_tiles[gi]
                        gi += 1
                        if t0 == 0:
                            # patch new-token k (column 0 of kt) and v (row 0
                            # of vv values; its mask col is host-staged 1.0)
                            nc.vector.tensor_copy(kvt[:, 0:1], knewT[:, j:j + 1])
                            nc.vector.tensor_copy(
                                kvt[0:1, w * P:w * P + D_V],
                                vnew[0:1, j * P:j * P + D_V],
                            )
                        for u0 in range(0, w, SUB):
                            u = min(SUB, w - u0)
                            sc = scps.tile([P, SUB * P], dtf, tag="sc")
                            for li in range(u):
                                nc.tensor.matmul(
                                    sc[:, li * P:(li + 1) * P],
                                    lhsT=kvt[:, (u0 + li) * P:(u0 + li + 1) * P],
                                    rhs=qjT[:],
                                    start=True, stop=True,
                                )
                            et = epool.tile([P, SUB * P], dtb, tag="et")
                            nc.scalar.activation(
                                et[:, 0:u * P], sc[:, 0:u * P], Exp,
                                bias=0.0, scale=SCALE,
                            )
                            for li in range(u):
                                t = t0 + u0 + li
                                nc.tensor.matmul(
                                    attn_ps[:],
                                    lhsT=et[:, li * P:(li + 1) * P],
                                    rhs=kvt[:, w * P + (u0 + li) * VW:w * P + (u0 + li + 1) * VW],
                                    start=(t == 0), stop=(t == Tj - 1),
                                    skip_group_check=True,
                                )
                    recip = spool.tile([P, 1], dtf, tag="recip")
                    nc.vector.reciprocal(recip[:], attn_ps[:, D_V:D_V + 1])
                    nc.vector.tensor_scalar_mul(
                        attn_cc[:, j * D_V:(j + 1) * D_V], attn_ps[:, 0:D_V], recip[:],
                    )
                    # stage this slot's columns immediately so the A2A2
                    # trigger only waits on the last slot's small store
                    nc.scalar.dma_start(
                        acc_in[:, j * D_V:(j + 1) * D_V],
                        attn_cc[:, j * D_V:(j + 1) * D_V])

            # wo tiles 14-21: gated on phase B completion (last slot's attn),
            # so their 5.2MB stream fills the otherwise-idle A2A2 window and
            # the phase-B kv stream gets the full HBM bandwidth.
            for i in range(8):
                dmy = wopoolB.tile([P, WO_HALF], dtb, tag="wt3")
                nc.vector.tensor_copy(
                    dmy[:, 0:1], attn_cc[:, (BPC - 1) * D_V:(BPC - 1) * D_V + 1])
            for kt in range(14, 22):
                half, k = divmod(kt, HPC)
                wt2 = wopoolB.tile([P, WO_HALF], dtb, tag="wt3")
                nc.gpsimd.dma_start(
                    wt2[:], wo_d.ap()[k * P:(k + 1) * P, half * WO_HALF:(half + 1) * WO_HALF])
                wo_tiles.append(wt2)

            nc.gpsimd.collective_compute(
                "AllToAll", mybir.AluOpType.bypass, replica_groups=rg,
                ins=[acc_in.opt()], outs=[acc_out.opt()],
            )

            # bm_all[(c,j), (hl,d)] = acc_out[c*16+hl, j*128+d]  (ACT ring)
            bm_all = cpool.tile([P, HPC * P], dtb)
            for cc in range(NC):
                nc.scalar.dma_start(
                    bm_all[cc * BPC:(cc + 1) * BPC, :]
                    .rearrange("j (hl d) -> j hl d", d=P),
                    acc_out.rearrange("(c hl) (j d) -> c j hl d", hl=HPC, d=P)[cc],
                )
            # rest of the wo stream (ring-paced by phase C consumption)
            for kt in range(22, 2 * HPC):
                half, k = divmod(kt, HPC)
                wt2 = wopoolA.tile([P, WO_HALF], dtb, tag="wt2")
                nc.gpsimd.dma_start(
                    wt2[:], wo_d.ap()[k * P:(k + 1) * P, half * WO_HALF:(half + 1) * WO_HALF])
                wo_tiles.append(wt2)

            # ---- phase C: Wo partial (this core's 16 heads, all 128 batches) ----
            with (
                tc.tile_pool(name="tps2", bufs=2, space="PSUM") as tps2,
                tc.tile_pool(name="wops", bufs=2, space="PSUM") as wops,
            ):
                lhs_tiles = []
                for hl in range(HPC):
                    lh_ps = tps2.tile([P, P], dtb, tag="tps2")
                    nc.tensor.transpose(lh_ps[:], bm_all[:, hl * P:(hl + 1) * P], idn[:])
                    lh = lhpool.tile([P, P], dtb, tag="lh")
                    nc.vector.tensor_copy(lh[:], lh_ps[:])
                    lhs_tiles.append(lh)
                QW = WO_HALF // 2                     # 1280
                for qp in range(4):
                    half, sub = divmod(qp, 2)
                    wo_ps = wops.tile([P, QW], dtf, tag="wop")
                    for kt in range(HPC):
                        wt2 = wo_tiles[half * HPC + kt]
                        for (c0, cw) in [(0, 512), (512, 512), (1024, 256)]:
                            nc.tensor.matmul(
                                wo_ps[:, c0:c0 + cw],
                                lhsT=lhs_tiles[kt][:],
                                rhs=wt2[:, sub * QW + c0:sub * QW + c0 + cw],
                                start=(kt == 0), stop=(kt == HPC - 1),
                            )
                    out_sb = opool.tile([P, QW], dtb, tag="osb")
                    nc.vector.tensor_copy(out_sb[:], wo_ps[:])
                    nc.sync.dma_start(
                        out_d.ap()[:, qp * QW:(qp + 1) * QW], out_sb[:],
                    )
            wo_stack.close()   # close wopool before the outer pools (LIFO)

    if do_compile:
        nc.compile()
    return nc


def kernel(hidden_states, k_cache, v_cache, Wq, Wkv, Wo, positions, slot_mapping,
           seq_lens):
    global LAST_RESULTS
    h = np.asarray(hidden_states, np.float32)[:, -1, :]        # [B, HID]
    k_cache = np.asarray(k_cache, np.float32)
    v_cache = np.asarray(v_cache, np.float32)
    Wq = np.asarray(Wq, np.float32)
    Wkv = np.asarray(Wkv, np.float32)
    Wo = np.asarray(Wo, np.float32)
    seq = np.asarray(seq_lens).astype(np.int64)

    # ---- schedule: sort batches by length, slot j holds ranks [8j, 8j+8) ----
    order = np.argsort(-seq, kind="stable")
    batch_of = np.empty((NC, BPC), np.int64)
    for j in range(BPC):
        for c in range(NC):
            batch_of[c, j] = order[NC * j + c]
    perm = batch_of.reshape(-1)                                # sigma (c-major)
    L = np.array([int(seq[order[NC * j]]) for j in range(BPC)])
    T = (L + P - 1) // P

    # kv blob layout: per slot, groups of up to GRP tiles; group = kt|vv cols
    kvoff = []
    KVCOLS = 0
    for j in range(BPC):
        groups = _groups_of(int(T[j]), KVCOLS)
        KVCOLS = groups[-1][2] + groups[-1][1] * TW
        kvoff.append(groups)

    key = (KVCOLS, tuple(int(t) for t in T))
    if key not in _PROGRAM_CACHE:
        _PROGRAM_CACHE.clear()
        _PROGRAM_CACHE[key] = _build_program(T, kvoff, KVCOLS)
    nc = _PROGRAM_CACHE[key]

    # ---- host staging ----
    # h^T in device layout: [p, k*128 + b] = h_sigma[b, k*128+p]; staged /GQ
    # because wq is staged xGQ (fp8 subnormal headroom) -> q_ps at true scale.
    h_sigma = (h[perm] / GQ).astype(BF16)                      # [128, HID]
    ht_stage = np.ascontiguousarray(
        h_sigma.reshape(P, HID // P, P).transpose(2, 1, 0).reshape(P, HID)
    )
    idn_np = np.eye(P, dtype=BF16)

    in_maps = []
    for c in range(NC):
        cols = [Wq[:, (HPC * c + hl) * (D + D_ROPE):(HPC * c + hl) * (D + D_ROPE) + D]
                for hl in range(HPC)]
        wq_flat = np.concatenate(cols + [Wkv[:, :256]], axis=1) * GQ  # [5120, 2304]
        # pack 2 k-tiles per super-tile: col = s*2*QCOLS + half*QCOLS + q,
        # value = wq_flat[(2s+half)*128 + p, q]; first NK16 supertiles bf16,
        # remaining NK8 supertiles fp8-e3m4 (clipped to its +-15.5 range).
        wq_pack = (
            wq_flat.reshape(NK16 + NK8, 2, P, QCOLS)
            .transpose(2, 0, 1, 3)
            .reshape(P, (NK16 + NK8) * 2 * QCOLS)
        )
        wq8_stage = np.ascontiguousarray(
            np.clip(wq_pack, -15.0, 15.0)).astype(E3M4)
        wo_stage = np.ascontiguousarray(
            Wo[HPC * c * D_V:(HPC * c + HPC) * D_V, :]
        ).astype(BF16)

        kv_stage = np.zeros((P, KVCOLS), BF16)
        for j in range(BPC):
            b = int(batch_of[c, j])
            S = int(seq[b])
            Tj = int(T[j])
            # kt block [128 d, Tj*128 s]: position 0 = new token (patched on
            # device), positions 1..S-1 = cache rows 0..S-2, rest 0.
            ktb = np.zeros((P, Tj * P), BF16)
            ktb[:, 1:S] = k_cache[b, :S - 1, :].T.astype(BF16)
            # vv block tiles [128 s, 129]: col 128 = valid mask
            vblk = np.zeros((Tj * P, VW), BF16)
            vblk[1:S, :D_V] = v_cache[b, :S - 1, :].astype(BF16)
            vblk[:S, D_V] = 1.0
            vtiles = vblk.reshape(Tj, P, VW)
            for (t0, w, off) in kvoff[j]:
                kw = w * P
                kv_stage[:, off:off + kw] = ktb[:, t0 * P:(t0 + w) * P]
                kv_stage[:, off + kw:off + kw + w * VW] = (
                    vtiles[t0:t0 + w].transpose(1, 0, 2).reshape(P, w * VW)
                )
        in_maps.append({
            "ht": ht_stage, "wq8": wq8_stage,
            "kv": kv_stage, "wo": wo_stage, "idn": idn_np,
        })

    trace = os.environ.get("BASS_KERNEL_TRACE", "0") == "1"
    if trace:
        _install_ntff_hook()
    kw = {}
    tcs = os.environ.get("BASS_TRACE_CORES", "")
    if trace and tcs:
        kw["trace_cores"] = list(range(int(tcs)))
    res = run_bass_kernel_spmd(nc, in_maps, core_ids=list(range(NC)), trace=trace, **kw)
    LAST_RESULTS = res

    out_sigma = np.zeros((P, HID), np.float32)
    for c in range(NC):
        out_sigma += res.results[c]["outp"].astype(np.float32)
    out_full = np.empty((B, HID), np.float32)
    out_full[perm] = out_sigma
    return out_full.reshape(B, 1, HID)
